# revision 14
# baseline (speedup 1.0000x reference)
"""Trainium2 Bass kernel for nn_ActLayer_49641232007349.

out[b,o] = sum_{i,f} norm(sin(freqs[f]*x[b,i] + phases[f])) * beta[f,o] * lamb[i,o] + bias[o]
with norm(s) = (s - mean_f) / sqrt(eps + var_f), B=8192, I=512, F=64, O=512.

Strategy (data-parallel over batch across 8 cores, 1024 rows each):
  out = sum_f S_f @ W_f  where  S_f = sin(w_f x + p_f)  [B,I]
        W_f = lamb * (c_f * beta[f,:])  [I,O],  c_f = 1/sqrt(eps+var_f)
  The mean-subtraction term is a rank-1 correction folded into a host-side
  bias:  bias_eff[o] = bias[o] - (sum_f c_f m_f beta[f,o]) * (sum_i lamb[i,o]).

Per core, per f: the vector engine computes r = rne((w_f x + p_f)/2pi) via an
fp32->int16 RNE output conversion, then d = (w_f/2pi) x - r (mixed-dtype
scalar_tensor_tensor); the scalar engine computes sin(2pi d + p_f) in
float32r (arg in [-pi,pi], inside the Sin spline range); the tensor engine
accumulates 32 [128x128]x[128x512] fp32r matmuls into 8 persistent PSUM banks
(one per 128-row output block). Weights are precomputed on the host (fp32r
rounded) and streamed from HBM.
"""
import sys
import math

sys.path.insert(0, "/opt/trn_rl_repo")

import numpy as np

import concourse.bacc as bacc
import concourse.mybir as mybir
import concourse.tile as tile
from concourse.bass_utils import run_bass_kernel_spmd

F32 = mybir.dt.float32
F32R = mybir.dt.float32r
I16 = mybir.dt.int16

N_CORES = 8
B, I, F, O = 8192, 512, 64, 512
BSH = B // N_CORES          # 1024 batch rows per core
IC = I // 128               # 4 i-chunks
BC = BSH // 128             # 8 b-chunks (one PSUM bank each)
EPS = 1e-3
BIG = 1.5 * 2.0**23         # 12582912.0: forces RNE-to-integer in fp32
TWO_PI = 2.0 * math.pi


def _round_fp32r(x: np.ndarray) -> np.ndarray:
    """Round fp32 to fp32r: 11 mantissa bits (RNE), low 12 bits zeroed.
    Verified bit-exact against trn2 hardware matmul behaviour."""
    u = np.ascontiguousarray(x, dtype=np.float32).view(np.uint32)
    low = u & np.uint32(0xFFF)
    base = u & ~np.uint32(0xFFF)
    lsb = (base >> np.uint32(12)) & np.uint32(1)
    round_up = (low > 0x800) | ((low == 0x800) & (lsb == 1))
    return (base + np.where(round_up, np.uint32(0x1000), np.uint32(0))).view(np.float32)


def _build(freqs_flat: np.ndarray, phases_flat: np.ndarray):
    """Build the per-core SPMD Bass module. freqs/phases values are baked in
    as tensor_scalar/activation immediates (phases also arrive per-partition
    via the bias2 DRAM tensor)."""
    nc = bacc.Bacc("TRN2", target_bir_lowering=False, debug=False)

    xt = nc.dram_tensor("xt", [128, IC * BSH], F32, kind="ExternalInput").ap()
    w = nc.dram_tensor("w", [F, 128, IC * O], F32R, kind="ExternalInput").ap()
    bias2 = nc.dram_tensor("bias2", [128, F], F32, kind="ExternalInput").ap()
    out = nc.dram_tensor("out", [BSH, O], F32, kind="ExternalOutput").ap()

    sub = mybir.AluOpType.subtract
    mult = mybir.AluOpType.mult
    add = mybir.AluOpType.add
    act_t = mybir.ActivationFunctionType

    with tile.TileContext(nc) as tc:
        with (
            tc.tile_pool(name="xpool", bufs=1) as xpool,
            tc.tile_pool(name="wpool", bufs=4) as wpool,
            tc.tile_pool(name="rpool", bufs=2) as rpool,
            tc.tile_pool(name="spool", bufs=2) as spool,
            tc.tile_pool(name="opool", bufs=2) as opool,
            tc.tile_pool(name="psum", bufs=1, space="PSUM") as pspool,
        ):
            xt_sb = xpool.tile([128, IC * BSH], F32, tag="xt")
            # chunked input DMA so f=0's chain can start on chunk 0 early
            for ic in range(IC):
                nc.sync.dma_start(xt_sb[:, ic * BSH:(ic + 1) * BSH],
                                  xt[:, ic * BSH:(ic + 1) * BSH])
            # small tiles live in opool so the matmul-feeding pools keep
            # large aligned bases (a 32B shift here costs ~45ns per matmul)
            b2_sb = opool.tile([128, F], F32, tag="b2")
            nc.sync.dma_start(b2_sb[:], bias2[:])
            # dummy 1-col Sin up front: walrus hoists the ACT table load
            # into the preamble instead of stalling the first real ACTIVATE
            warm = opool.tile([128, 1], F32, tag="warm")
            nc.vector.memset(warm[:], 0.0)
            nc.scalar.activation(warm[:], warm[:], act_t.Sin, bias=0.0, scale=1.0)

            psum_tiles = [
                pspool.tile([128, O], F32, tag=f"ps{bc}", name=f"ps{bc}")
                for bc in range(BC)
            ]

            for f in range(F):
                sf = float(freqs_flat[f]) / TWO_PI
                pf_turn = float(phases_flat[f]) / TWO_PI

                w_sb = wpool.tile([128, IC * O], F32R, tag="w")
                if f == 0:
                    for ic in range(IC):
                        nc.sync.dma_start(w_sb[:, ic * O:(ic + 1) * O],
                                          w[f][:, ic * O:(ic + 1) * O])
                else:
                    nc.sync.dma_start(w_sb[:], w[f])

                # r = rne(w x/2pi + p/2pi) via fp32->int16 RNE conversion,
                # then d = (x * w/2pi) - r (|d + p/2pi| <= 0.5),
                # then s = sin(2pi d + p) -> float32r.
                # f=0 runs chunked per i-block so the PE starts early.
                rt = rpool.tile([128, IC * BSH], I16, tag="rt")
                dd = rpool.tile([128, IC * BSH], F32, tag="dd")
                ss = spool.tile([128, IC * BSH], F32R, tag="ss")
                chunks = [(0, 512), (512, 1024), (1024, 2048), (2048, 4096)] \
                    if f == 0 else [(0, IC * BSH)]
                for c0, c1 in chunks:
                    if f in (1, 2):
                        # early phase: PE is still ramping, so GPSIMD can take
                        # the round op without the SBUF-port contention that
                        # makes it a loss in steady state
                        nc.gpsimd.tensor_scalar(rt[:, c0:c1], xt_sb[:, c0:c1],
                                                sf, pf_turn, mult, add)
                    else:
                        nc.vector.tensor_scalar(rt[:, c0:c1], xt_sb[:, c0:c1],
                                                sf, pf_turn, mult, add)
                    nc.vector.scalar_tensor_tensor(dd[:, c0:c1], xt_sb[:, c0:c1],
                                                   sf, rt[:, c0:c1], mult, sub)
                    nc.scalar.activation(ss[:, c0:c1], dd[:, c0:c1], act_t.Sin,
                                         bias=b2_sb[:, f:f + 1], scale=TWO_PI)

                if f == 0:
                    # walk (ic, bc) in ss-column order so each chunk's matmuls
                    # issue as soon as its sin lands
                    for blk in range(IC * BC):
                        ic, bc = blk // BC, blk % BC
                        nc.tensor.matmul(
                            psum_tiles[bc][:],
                            lhsT=ss[:, ic * BSH + bc * 128: ic * BSH + bc * 128 + 128],
                            rhs=w_sb[:, ic * O: (ic + 1) * O],
                            start=(ic == 0),
                            stop=False,
                        )
                else:
                    for ic in range(IC):
                        for bc in range(BC):
                            nc.tensor.matmul(
                                psum_tiles[bc][:],
                                lhsT=ss[:, ic * BSH + bc * 128: ic * BSH + bc * 128 + 128],
                                rhs=w_sb[:, ic * O: (ic + 1) * O],
                                start=False,
                                stop=(f == F - 1 and ic == IC - 1),
                            )

            for bc in range(BC):
                ot = opool.tile([128, O], F32, tag=f"ot{bc % 2}")
                if bc % 2 == 0:
                    nc.vector.tensor_copy(ot[:], psum_tiles[bc][:])
                else:
                    nc.scalar.copy(ot[:], psum_tiles[bc][:])
                nc.sync.dma_start(out[bc * 128: (bc + 1) * 128, :], ot[:])

    nc.finalize()
    return nc


def kernel(x, freqs, phases, beta, lamb, bias, _trace=False):
    x = np.ascontiguousarray(x, dtype=np.float32)
    wf = np.asarray(freqs, dtype=np.float32).reshape(-1)      # [F]
    ph = np.asarray(phases, dtype=np.float32).reshape(-1)     # [F]
    beta = np.asarray(beta, dtype=np.float32)                 # [F, O]
    lamb = np.asarray(lamb, dtype=np.float32)                 # [I, O]
    bias = np.asarray(bias, dtype=np.float32)                 # [O]

    # normalization constants (fp64 on host for accuracy)
    wf64, ph64 = wf.astype(np.float64), ph.astype(np.float64)
    mean = np.exp(-0.5 * wf64**2) * np.sin(ph64)                        # [F]
    var = 0.5 - 0.5 * np.exp(-2.0 * wf64**2) * np.cos(2.0 * ph64) - mean**2
    cf = 1.0 / np.sqrt(EPS + var)                                       # [F]

    # per-f weights W_f = lamb * (c_f beta_f), laid out [f, i_in, ic, o]
    w_full = lamb[None, :, :] * (cf[:, None] * beta.astype(np.float64))[:, None, :]
    w_full = w_full.astype(np.float32).reshape(F, IC, 128, O).transpose(0, 2, 1, 3)
    w_host = _round_fp32r(np.ascontiguousarray(w_full).reshape(F, 128, IC * O))

    # rank-1 mean correction folded into host-side bias
    const_o = (cf * mean) @ beta.astype(np.float64) * lamb.astype(np.float64).sum(0)
    bias_eff = (bias.astype(np.float64) - const_o).astype(np.float32)   # [O]

    b2 = np.broadcast_to(ph, (128, F)).copy()

    nc = _build(wf, ph)

    in_maps = []
    for c in range(N_CORES):
        xs = x[c * BSH: (c + 1) * BSH]                        # [BSH, I]
        xtc = np.ascontiguousarray(
            xs.reshape(BSH, IC, 128).transpose(2, 1, 0).reshape(128, IC * BSH)
        )
        in_maps.append({"xt": xtc, "w": w_host, "bias2": b2})

    res = None
    for attempt in range(3):
        try:
            res = run_bass_kernel_spmd(nc, in_maps, core_ids=list(range(N_CORES)),
                                       trace=_trace)
            break
        except Exception:
            # transient NRT_EXEC_UNIT_UNRECOVERABLE wedges clear on reload
            if attempt == 2:
                raise
            import time as _time
            _time.sleep(5.0)

    out = np.empty((B, O), dtype=np.float32)
    for c in range(N_CORES):
        out[c * BSH: (c + 1) * BSH] = res.results[c]["out"]
    out += bias_eff[None, :]
    if _trace:
        return out, res
    return out
